# revision 1
# baseline (speedup 1.0000x reference)
"""Trainium2 Bass kernel for nn_Attention_21088289423660 (sparse_attention).

Reference computation (per token t = (b, n, m), feature dim D=256):
    kh = Wk^T k_t                  (feature-major: [e, t])
    qh = Wq^T q_t
    v  = Wv^T kh = (Wk Wv)^T k_t   <- folded on host: Wkv = Wk @ Wv
    S  = kh - qh + pos_t           <- Wqn = -Wq accumulated in PSUM
    attn = sigmoid(W2^T relu(W1^T S + b1) + b2)      (mask is all-ones)
    out  = Wo^T ((v + pos_t) * attn) (+ bo on host)  (already feature-major)

Sharding: data-parallel over 8 cores; core c handles batch b=c//2 and
N-half (c%2) -> 16384 tokens/core, weights replicated.

Compute dtype: bf16 (PSUM accumulation fp32), device output bf16,
host adds bo and widens to fp32.
"""

import os
import sys

for _p in (
    "/root/.axon_site",
    "/root/.axon_site/_ro/trn_rl_repo",
    "/root/.axon_site/_ro/pypackages",
    "/opt/trn_rl_repo",
):
    if os.path.isdir(_p) and _p not in sys.path:
        sys.path.append(_p)

import numpy as np
import ml_dtypes
from contextlib import ExitStack

import concourse.bass as bass
import concourse.tile as tile
import concourse.bacc as bacc
from concourse import mybir
from concourse import bass_utils

BF16 = ml_dtypes.bfloat16
FP8 = ml_dtypes.float8_e4m3
FP8_S = os.environ.get("KERNEL_FP8_S", "0") == "1"

B, DIM, N, M = 4, 256, 4096, 8
NCORES = 8
NT = (B * N * M) // NCORES          # tokens per core = 16384
P = 128                              # partitions
NDC = DIM // P                       # d-chunks = 2
CHUNK = 4096                         # tokens per DMA chunk
FD = 512                             # tokens per matmul tile
F32 = mybir.dt.float32
BF = mybir.dt.bfloat16
F8 = mybir.dt.float8e4

_CACHED_NC = None


def _build_nc():
    """Build and compile the per-core Bass program (SPMD, identical on all cores).

    Software-pipelined at depth 5: every stage consumes tiles produced at
    least one 512-token iteration earlier, so no engine waits on an
    intra-iteration chain.
    """
    nc = bacc.Bacc("TRN2", target_bir_lowering=False, debug=False)

    if not FP8_S:
        q_d = nc.dram_tensor("qs", (NDC, P, NT), BF, kind="ExternalInput").ap()
        wk_d = nc.dram_tensor("wk", (NDC, P, DIM), BF, kind="ExternalInput").ap()
        wqn_d = nc.dram_tensor("wqn", (NDC, P, DIM), BF, kind="ExternalInput").ap()
    else:
        q8_d = nc.dram_tensor("q8", (NDC, P, NT), F8, kind="ExternalInput").ap()
        k8_d = nc.dram_tensor("k8", (NDC, P, NT), F8, kind="ExternalInput").ap()
        wk8_d = nc.dram_tensor("wk8", (P, NDC, DIM), F8, kind="ExternalInput").ap()
        wqn8_d = nc.dram_tensor("wqn8", (P, NDC, DIM), F8, kind="ExternalInput").ap()
    k_d = nc.dram_tensor("ks", (NDC, P, NT), BF, kind="ExternalInput").ap()
    pos_d = nc.dram_tensor("poss", (NDC, P, NT), BF, kind="ExternalInput").ap()
    wkv_d = nc.dram_tensor("wkv", (NDC, P, DIM), BF, kind="ExternalInput").ap()
    w1_d = nc.dram_tensor("w1", (NDC, P, DIM // 2), BF, kind="ExternalInput").ap()
    w2_d = nc.dram_tensor("w2", (P, DIM), BF, kind="ExternalInput").ap()
    wo_d = nc.dram_tensor("wo", (NDC, P, DIM), BF, kind="ExternalInput").ap()
    b1_d = nc.dram_tensor("b1", (P, 1), F32, kind="ExternalInput").ap()
    b2_d = nc.dram_tensor("b2", (NDC, P, 1), F32, kind="ExternalInput").ap()
    out_d = nc.dram_tensor("out", (NDC, P, NT), BF, kind="ExternalOutput").ap()

    k_r = k_d.rearrange("c p t -> p c t")
    pos_r = pos_d.rearrange("c p t -> p c t")
    out_r = out_d.rearrange("c p t -> p c t")
    if FP8_S:
        q8_r = q8_d.rearrange("c p t -> p c t")
        k8_r = k8_d.rearrange("c p t -> p c t")
    else:
        q_r = q_d.rearrange("c p t -> p c t")

    AF = mybir.ActivationFunctionType
    DRmode = mybir.MatmulPerfMode.DoubleRow
    n_chunks = NT // CHUNK
    ipc = CHUNK // FD                   # iters per chunk
    n_total = NT // FD                  # global iterations

    with tile.TileContext(nc) as tc, ExitStack() as ctx:
        wpool = ctx.enter_context(tc.tile_pool(name="wpool", bufs=1))
        iopool = ctx.enter_context(tc.tile_pool(name="iopool", bufs=2))
        mid = ctx.enter_context(tc.tile_pool(name="mid", bufs=3))
        pp = ctx.enter_context(tc.tile_pool(name="pp", bufs=1, space="PSUM"))

        # --- weights / biases resident in SBUF ---
        wk_t, wqn_t, wkv_t, wo_t, w1_t = [], [], [], [], []
        # S-path weights first, on the fast sync ring (the first matmuls wait on them)
        if FP8_S:
            wk8_t = wpool.tile([P, NDC, DIM], F8, tag="wk8", name="wk8")
            nc.gpsimd.dma_start(wk8_t[:], wk8_d[:])
            wqn8_t = wpool.tile([P, NDC, DIM], F8, tag="wqn8", name="wqn8")
            nc.gpsimd.dma_start(wqn8_t[:], wqn8_d[:])
        else:
            for c in range(NDC):
                wt = wpool.tile([P, DIM], BF, tag=f"wk{c}", name=f"wk{c}")
                nc.scalar.dma_start(wt[:], wk_d[c])
                wk_t.append(wt)
            for c in range(NDC):
                wt = wpool.tile([P, DIM], BF, tag=f"wqn{c}", name=f"wqn{c}")
                nc.scalar.dma_start(wt[:], wqn_d[c])
                wqn_t.append(wt)
        for c in range(NDC):
            wt = wpool.tile([P, DIM], BF, tag=f"wkv{c}", name=f"wkv{c}")
            nc.gpsimd.dma_start(wt[:], wkv_d[c])
            wkv_t.append(wt)
            wt = wpool.tile([P, DIM], BF, tag=f"wo{c}", name=f"wo{c}")
            nc.gpsimd.dma_start(wt[:], wo_d[c])
            wo_t.append(wt)
            wt = wpool.tile([P, DIM // 2], BF, tag=f"w1{c}", name=f"w1{c}")
            nc.gpsimd.dma_start(wt[:], w1_d[c])
            w1_t.append(wt)
        w2_t = wpool.tile([P, DIM], BF, tag="w2", name="w2")
        nc.gpsimd.dma_start(w2_t[:], w2_d[:])
        b1_t = wpool.tile([P, 1], F32, tag="b1", name="b1")
        nc.gpsimd.dma_start(b1_t[:], b1_d[:])
        b2_t = []
        for c in range(NDC):
            bt = wpool.tile([P, 1], F32, tag=f"b2{c}", name=f"b2{c}")
            nc.gpsimd.dma_start(bt[:], b2_d[c])
            b2_t.append(bt)

        # Hoist the ACT spline-table load into the DMA head: a dummy relu +
        # sigmoid on a memset tile triggers ACT_TABLE_LOAD (~2.6us) before
        # the pipelined body needs it.
        dum_in = wpool.tile([P, 1], F32, tag="dum_in", name="dum_in")
        nc.gpsimd.memset(dum_in[:], 0.0)
        dum_out = wpool.tile([P, 1], BF, tag="dum_out", name="dum_out")
        nc.scalar.activation(dum_out[:], dum_in[:], AF.Relu)
        nc.scalar.activation(dum_out[:], dum_in[:], AF.Sigmoid)

        io = {}        # chunk -> dict of io tiles
        st = {}        # global iter -> dict of stage tiles

        def load_chunk(ci):
            csl = bass.ts(ci, CHUNK)
            kt = iopool.tile([P, NDC, CHUNK], BF, tag="kt", bufs=3, name="kt")
            post = iopool.tile([P, NDC, CHUNK], BF, tag="post", bufs=2, name="post")
            t = {"kt": kt, "post": post}
            if FP8_S:
                t["q8t"] = iopool.tile([P, NDC, CHUNK], F8, tag="q8t", bufs=3, name="q8t")
                t["k8t"] = iopool.tile([P, NDC, CHUNK], F8, tag="k8t", bufs=3, name="k8t")
                srcs = [(t["k8t"], k8_r, nc.scalar), (t["q8t"], q8_r, nc.scalar),
                        (kt, k_r, nc.sync), (post, pos_r, nc.sync)]
            else:
                t["qt"] = iopool.tile([P, NDC, CHUNK], BF, tag="qt", bufs=3, name="qt")
                srcs = [(kt, k_r, nc.sync), (t["qt"], q_r, nc.scalar), (post, pos_r, nc.sync)]
            if ci == 0:
                for si, sl in enumerate((slice(0, FD), slice(FD, 2 * FD),
                                         slice(2 * FD, CHUNK))):
                    for tdst, tsrc, eng in srcs:
                        if si < 2:
                            e = nc.scalar if tdst is post else nc.sync
                        else:
                            e = eng
                        e.dma_start(tdst[:, :, sl], tsrc[:, :, sl])
            else:
                for tdst, tsrc, eng in srcs:
                    eng.dma_start(tdst[:], tsrc[:, :, csl])
            t["outt"] = iopool.tile([P, NDC, CHUNK], BF, tag="outt", bufs=2, name="outt")
            io[ci] = t

        def stage1(j):
            ci, it = divmod(j, ipc)
            t = io[ci]
            tsl = bass.ts(it, FD)
            Sw = pp.tile([P, NDC, FD], F32, tag="S", bufs=1, name="Sw")
            Vw = pp.tile([P, NDC, FD], F32, tag="V", bufs=1, name="Vw")
            for e in range(NDC):
                esl = bass.ts(e, P)
                if FP8_S:
                    nc.tensor.matmul(Sw[:, e, :], wk8_t[:, :, esl], t["k8t"][:, :, tsl],
                                     start=True, stop=False, perf_mode=DRmode)
                    nc.tensor.matmul(Sw[:, e, :], wqn8_t[:, :, esl], t["q8t"][:, :, tsl],
                                     start=False, stop=True, perf_mode=DRmode)
                else:
                    nc.tensor.matmul(Sw[:, e, :], wk_t[0][:, esl], t["kt"][:, 0, tsl],
                                     start=True, stop=False)
                    nc.tensor.matmul(Sw[:, e, :], wk_t[1][:, esl], t["kt"][:, 1, tsl],
                                     start=False, stop=False)
                    nc.tensor.matmul(Sw[:, e, :], wqn_t[0][:, esl], t["qt"][:, 0, tsl],
                                     start=False, stop=False)
                    nc.tensor.matmul(Sw[:, e, :], wqn_t[1][:, esl], t["qt"][:, 1, tsl],
                                     start=False, stop=True)
            for e in range(NDC):
                esl = bass.ts(e, P)
                nc.tensor.matmul(Vw[:, e, :], wkv_t[0][:, esl], t["kt"][:, 0, tsl],
                                 start=True, stop=False)
                nc.tensor.matmul(Vw[:, e, :], wkv_t[1][:, esl], t["kt"][:, 1, tsl],
                                 start=False, stop=True)
            ap_t = mid.tile([P, NDC, FD], BF, tag="ap", bufs=4, name="ap_t")
            nc.vector.tensor_add(ap_t[:], Sw[:], t["post"][:, :, tsl])
            vp_t = mid.tile([P, NDC, FD], BF, tag="vp", bufs=6, name="vp_t")
            nc.vector.tensor_add(vp_t[:], Vw[:], t["post"][:, :, tsl])
            st[j] = {"ap": ap_t, "vp": vp_t}

        def mlpA(j):
            s = st[j]
            h1p = pp.tile([P, FD], F32, tag="mlp", bufs=2, name="h1p")
            nc.tensor.matmul(h1p[:], w1_t[0][:], s["ap"][:, 0, :], start=True, stop=False)
            nc.tensor.matmul(h1p[:], w1_t[1][:], s["ap"][:, 1, :], start=False, stop=True)
            h1r = mid.tile([P, FD], BF, tag="h1r", bufs=4, name="h1r")
            nc.scalar.activation(h1r[:], h1p[:], AF.Relu, bias=b1_t[:, 0:1])
            s["h1r"] = h1r

        def mlpB(j):
            s = st[j]
            at_t = mid.tile([P, NDC, FD], BF, tag="at", bufs=4, name="at_t")
            for e in range(NDC):
                esl = bass.ts(e, P)
                a2p = pp.tile([P, FD], F32, tag="mlp", bufs=2, name="a2p")
                nc.tensor.matmul(a2p[:], w2_t[:, esl], s["h1r"][:], start=True, stop=True)
                nc.scalar.activation(at_t[:, e, :], a2p[:], AF.Sigmoid,
                                     bias=b2_t[e][:, 0:1])
            s["at"] = at_t

        def gate(j):
            s = st[j]
            g_t = mid.tile([P, NDC, FD], BF, tag="g", bufs=4, name="g_t")
            nc.gpsimd.tensor_mul(g_t[:, 0, :], s["vp"][:, 0, :], s["at"][:, 0, :])
            nc.vector.tensor_mul(g_t[:, 1, :], s["vp"][:, 1, :], s["at"][:, 1, :])
            s["g"] = g_t

        def out(j):
            ci, it = divmod(j, ipc)
            t = io[ci]
            tsl = bass.ts(it, FD)
            s = st[j]
            for e in range(NDC):
                esl = bass.ts(e, P)
                xo = pp.tile([P, FD], F32, tag="xo", bufs=2, name="xo")
                nc.tensor.matmul(xo[:], wo_t[0][:, esl], s["g"][:, 0, :],
                                 start=True, stop=False)
                nc.tensor.matmul(xo[:], wo_t[1][:, esl], s["g"][:, 1, :],
                                 start=False, stop=True)
                if e == 0:
                    nc.vector.tensor_copy(t["outt"][:, e, tsl], xo[:])
                else:
                    nc.scalar.copy(t["outt"][:, e, tsl], xo[:])
            del st[j]
            hh = CHUNK // 2
            if it == ipc // 2 - 1:
                nc.sync.dma_start(out_r[:, :, ci * CHUNK:ci * CHUNK + hh],
                                  t["outt"][:, :, 0:hh])
            elif it == ipc - 1:
                if ci == NT // CHUNK - 1:
                    qq = 3 * CHUNK // 4
                    nc.sync.dma_start(out_r[:, :, ci * CHUNK + hh:ci * CHUNK + qq],
                                      t["outt"][:, :, hh:qq])
                    nc.sync.dma_start(out_r[:, :, ci * CHUNK + qq:(ci + 1) * CHUNK],
                                      t["outt"][:, :, qq:CHUNK])
                else:
                    nc.sync.dma_start(out_r[:, :, ci * CHUNK + hh:(ci + 1) * CHUNK],
                                      t["outt"][:, :, hh:CHUNK])

        for j in range(n_total + 4):
            if j >= 3 and j - 3 < n_total:
                gate(j - 3)
            if j < n_total:
                if j % ipc == 0:
                    load_chunk(j // ipc)
                stage1(j)
            if j >= 1 and j - 1 < n_total:
                mlpA(j - 1)
            if j >= 4 and j - 4 < n_total:
                out(j - 4)
            if j >= 2 and j - 2 < n_total:
                mlpB(j - 2)

    nc.compile()
    return nc


def _get_nc():
    global _CACHED_NC
    if _CACHED_NC is None:
        _CACHED_NC = _build_nc()
    return _CACHED_NC


def _prep_in_maps(q, k, pos, Wq, Wk, Wv, W1, b1, W2, b2, Wo, bo):
    q = np.asarray(q, dtype=np.float32)
    k = np.asarray(k, dtype=np.float32)
    pos = np.asarray(pos, dtype=np.float32)

    weights = {
        "wkv": np.ascontiguousarray(
            (np.asarray(Wk, np.float32) @ np.asarray(Wv, np.float32)).astype(BF16)
        ).reshape(NDC, P, DIM),
        "w1": np.ascontiguousarray(np.asarray(W1, np.float32).astype(BF16)).reshape(NDC, P, DIM // 2),
        "w2": np.ascontiguousarray(np.asarray(W2, np.float32).astype(BF16)).reshape(P, DIM),
        "wo": np.ascontiguousarray(np.asarray(Wo, np.float32).astype(BF16)).reshape(NDC, P, DIM),
        "b1": np.asarray(b1, np.float32).reshape(P, 1),
        "b2": np.asarray(b2, np.float32).reshape(NDC, P, 1),
    }
    Wk32 = np.asarray(Wk, np.float32)
    Wqn32 = -np.asarray(Wq, np.float32)
    if FP8_S:
        weights["wk8"] = np.ascontiguousarray(
            Wk32.reshape(NDC, P, DIM).transpose(1, 0, 2).astype(FP8))
        weights["wqn8"] = np.ascontiguousarray(
            Wqn32.reshape(NDC, P, DIM).transpose(1, 0, 2).astype(FP8))
    else:
        weights["wk"] = np.ascontiguousarray(Wk32.astype(BF16)).reshape(NDC, P, DIM)
        weights["wqn"] = np.ascontiguousarray(Wqn32.astype(BF16)).reshape(NDC, P, DIM)

    nhalf = N // 2
    in_maps = []
    for c in range(NCORES):
        b = c // 2
        n0 = (c % 2) * nhalf
        qs = q[b, :, n0:n0 + nhalf, :].reshape(DIM, NT)
        ks = k[b, :, n0:n0 + nhalf, :].reshape(DIM, NT)
        ps = np.ascontiguousarray(
            pos[b, n0:n0 + nhalf].reshape(NT, DIM).T
        ).astype(BF16)
        m = dict(weights)
        m["ks"] = ks.astype(BF16).reshape(NDC, P, NT)
        m["poss"] = ps.reshape(NDC, P, NT)
        if FP8_S:
            m["q8"] = qs.astype(FP8).reshape(NDC, P, NT)
            m["k8"] = ks.astype(FP8).reshape(NDC, P, NT)
        else:
            m["qs"] = qs.astype(BF16).reshape(NDC, P, NT)
        in_maps.append(m)
    return in_maps


def _run(in_maps, trace=False, **kwargs):
    nc = _get_nc()
    return bass_utils.run_bass_kernel_spmd(
        nc, in_maps, core_ids=list(range(NCORES)), trace=trace, **kwargs
    )


def _assemble(results, bo, mask):
    bo = np.asarray(bo, np.float32)
    out = np.empty((B, DIM, N, M), dtype=np.float32)
    nhalf = N // 2
    for c in range(NCORES):
        b = c // 2
        n0 = (c % 2) * nhalf
        r = results[c]["out"].reshape(DIM, nhalf, M).astype(np.float32)
        r += bo[:, None, None]
        out[b, :, n0:n0 + nhalf, :] = r
    mask = np.asarray(mask)
    if not np.all(mask != 0):
        # masked positions: sigmoid(-1e9)=0 -> x=0 -> out = bo
        zb, zn, zm = np.nonzero(mask[..., 0] == 0)
        out[zb, :, zn, zm] = bo[None, :]
    return out


def kernel(q, k, pos, mask, Wq, Wk, Wv, W1, b1, W2, b2, Wo, bo):
    in_maps = _prep_in_maps(q, k, pos, Wq, Wk, Wv, W1, b1, W2, b2, Wo, bo)
    res = _run(in_maps)
    return _assemble(res.results, bo, mask)


def install_profile_hook():
    """Register the axon NTFF profiling hook (antenv.axon_hooks shim) so
    run_bass_kernel_spmd(trace=True) yields exec_time_ns + perfetto trace."""
    import types

    try:
        import antenv.axon_hooks  # noqa: F401
        return True
    except ImportError:
        pass
    try:
        from trn_agent_boot.trn_boot import _ntff_profile_via_ctypes
    except ImportError:
        return False
    hook = _ntff_profile_via_ctypes("/opt/axon/libaxon_pjrt.so")
    if hook is None:
        return False
    mod = types.ModuleType("antenv.axon_hooks")
    mod.get_axon_ntff_profile_hook = lambda: hook
    mod.set_axon_ntff_profile_hook = lambda h: None
    import antenv

    sys.modules["antenv.axon_hooks"] = mod
    antenv.axon_hooks = mod
    # artifact upload has no share reachable from this container
    bass_utils.upload_artifacts = lambda tmpdir: tmpdir
    return True



# revision 3
# speedup vs baseline: 1.1580x; 1.1580x over previous
"""Trainium2 Bass kernel for nn_Attention_21088289423660 (sparse_attention).

Reference computation (per token t = (b, n, m), feature dim D=256):
    kh = Wk^T k_t ; qh = Wq^T q_t ; v = Wv^T kh
    S  = kh - qh + pos_t
    attn = sigmoid(W2^T relu(W1^T S + b1) + b2)      (mask is all-ones)
    out  = Wo^T ((v + pos_t) * attn) + bo

Folded algebra (S is never materialized):
    h1  = A^T k + Bn^T q + W1^T pos + b1    A = Wk@W1, Bn = -Wq@W1   (6 MMs)
    v   = Wkv^T k                            Wkv = Wk@Wv              (4 MMs)
    h2  = W2^T relu(h1)                                               (2 MMs)
    attn = sigmoid(h2 + b2)
    out  = Wo^T ((v + pos) * attn)                                    (4 MMs)
16 matmul-columns/token vs 20 unfused.

Sharding: data-parallel over 8 cores; core c handles batch b=c//2 and
N-half (c%2) -> 16384 tokens/core, weights replicated.

Compute dtype: bf16 (PSUM accumulation fp32), device output bf16,
host adds bo and widens to fp32.
"""

import os
import sys

for _p in (
    "/root/.axon_site",
    "/root/.axon_site/_ro/trn_rl_repo",
    "/root/.axon_site/_ro/pypackages",
    "/opt/trn_rl_repo",
):
    if os.path.isdir(_p) and _p not in sys.path:
        sys.path.append(_p)

import numpy as np
import ml_dtypes
from contextlib import ExitStack

import concourse.bass as bass
import concourse.tile as tile
import concourse.bacc as bacc
from concourse import mybir
from concourse import bass_utils

BF16 = ml_dtypes.bfloat16

B, DIM, N, M = 4, 256, 4096, 8
NCORES = 8
NT = (B * N * M) // NCORES          # tokens per core = 16384
P = 128                              # partitions
NDC = DIM // P                       # d-chunks = 2
CHUNK = 2048                         # tokens per DMA chunk
FD = 512                             # tokens per matmul tile
F32 = mybir.dt.float32
BF = mybir.dt.bfloat16

WARMUP_MMS = int(os.environ.get("KERNEL_WARMUP_MMS", "14"))

_CACHED_NC = None


def _build_nc():
    """Build and compile the per-core Bass program (SPMD, identical on all cores).

    Software-pipelined at depth 5; dummy warm-up matmuls run during the DMA
    head so the PE HAM clock gate is at 8/8 before real matmuls start.
    """
    nc = bacc.Bacc("TRN2", target_bir_lowering=False, debug=False)

    q_d = nc.dram_tensor("qs", (NDC, P, NT), BF, kind="ExternalInput").ap()
    k_d = nc.dram_tensor("ks", (NDC, P, NT), BF, kind="ExternalInput").ap()
    pos_d = nc.dram_tensor("poss", (NDC, P, NT), BF, kind="ExternalInput").ap()
    a_d = nc.dram_tensor("wa", (NDC, P, P), BF, kind="ExternalInput").ap()
    bn_d = nc.dram_tensor("wbn", (NDC, P, P), BF, kind="ExternalInput").ap()
    w1_d = nc.dram_tensor("w1", (NDC, P, P), BF, kind="ExternalInput").ap()
    wkv_d = nc.dram_tensor("wkv", (NDC, P, DIM), BF, kind="ExternalInput").ap()
    w2_d = nc.dram_tensor("w2", (P, DIM), BF, kind="ExternalInput").ap()
    wo_d = nc.dram_tensor("wo", (NDC, P, DIM), BF, kind="ExternalInput").ap()
    b1_d = nc.dram_tensor("b1", (P, 1), F32, kind="ExternalInput").ap()
    b2_d = nc.dram_tensor("b2", (NDC, P, 1), F32, kind="ExternalInput").ap()
    out_d = nc.dram_tensor("out", (NDC, P, NT), BF, kind="ExternalOutput").ap()

    k_r = k_d.rearrange("c p t -> p c t")
    q_r = q_d.rearrange("c p t -> p c t")
    pos_r = pos_d.rearrange("c p t -> p c t")
    out_r = out_d.rearrange("c p t -> p c t")

    AF = mybir.ActivationFunctionType
    n_chunks = NT // CHUNK              # 8
    ipc = CHUNK // FD                   # iters per chunk = 4
    n_total = NT // FD                  # global iterations = 32

    with tile.TileContext(nc) as tc, ExitStack() as ctx:
        wpool = ctx.enter_context(tc.tile_pool(name="wpool", bufs=1))
        iopool = ctx.enter_context(tc.tile_pool(name="iopool", bufs=2))
        mid = ctx.enter_context(tc.tile_pool(name="mid", bufs=3))
        pp = ctx.enter_context(tc.tile_pool(name="pp", bufs=1, space="PSUM"))

        # --- warm-up scratch (no DMA deps; PE can start immediately) ---
        wu_w = wpool.tile([P, P], BF, tag="wu_w", name="wu_w")
        nc.gpsimd.memset(wu_w[:], 0.0)
        wu_in = wpool.tile([P, 256], BF, tag="wu_in", name="wu_in")
        nc.gpsimd.memset(wu_in[:], 0.0)

        # --- weights / biases resident in SBUF ---
        # h1-path weights first on the fast sync ring (first matmuls wait on them)
        a_t = wpool.tile([P, NDC, P], BF, tag="wa", name="wa")
        nc.sync.dma_start(a_t[:], a_d.rearrange("c p e -> p c e"))
        bn_t = wpool.tile([P, NDC, P], BF, tag="wbn", name="wbn")
        nc.sync.dma_start(bn_t[:], bn_d.rearrange("c p e -> p c e"))
        w1_t = wpool.tile([P, NDC, P], BF, tag="w1", name="w1")
        nc.sync.dma_start(w1_t[:], w1_d.rearrange("c p e -> p c e"))
        wkv_t = wpool.tile([P, NDC, DIM], BF, tag="wkv", name="wkv")
        nc.gpsimd.dma_start(wkv_t[:], wkv_d.rearrange("c p e -> p c e"))
        wo_t = wpool.tile([P, NDC, DIM], BF, tag="wo", name="wo")
        nc.gpsimd.dma_start(wo_t[:], wo_d.rearrange("c p e -> p c e"))
        w2_t = wpool.tile([P, DIM], BF, tag="w2", name="w2")
        nc.scalar.dma_start(w2_t[:], w2_d[:])
        b1_t = wpool.tile([P, 1], F32, tag="b1", name="b1")
        nc.scalar.dma_start(b1_t[:], b1_d[:])
        b2_t = []
        for c in range(NDC):
            bt = wpool.tile([P, 1], F32, tag=f"b2{c}", name=f"b2{c}")
            nc.scalar.dma_start(bt[:], b2_d[c])
            b2_t.append(bt)

        # Hoist the ACT spline-table load into the DMA head: a dummy relu +
        # sigmoid on a memset tile triggers ACT_TABLE_LOAD (~2.6us) before
        # the pipelined body needs it.
        dum_out = wpool.tile([P, 1], BF, tag="dum_out", name="dum_out")
        nc.scalar.activation(dum_out[:], wu_w[:, 0:1], AF.Relu)
        nc.scalar.activation(dum_out[:], wu_w[:, 0:1], AF.Sigmoid)

        # --- HAM warm-up: dummy matmuls on zero tiles while DMAs land ---
        for _ in range(WARMUP_MMS):
            wu_ps = pp.tile([P, NDC, FD], F32, tag="xo", bufs=1, name="wu_ps")
            nc.tensor.matmul(wu_ps[:, 0, 0:256], wu_w[:], wu_in[:],
                             start=True, stop=True)

        io = {}        # chunk -> dict of io tiles
        st = {}        # global iter -> dict of stage tiles

        def load_chunk(ci):
            csl = bass.ts(ci, CHUNK)
            kt = iopool.tile([P, NDC, CHUNK], BF, tag="kt", bufs=3, name="kt")
            qt = iopool.tile([P, NDC, CHUNK], BF, tag="qt", bufs=3, name="qt")
            post = iopool.tile([P, NDC, CHUNK], BF, tag="post", bufs=3, name="post")
            if ci == 0:
                # fine slices so s1(0) can start after ~512 tokens arrived
                for si in range(ipc):
                    sl = bass.ts(si, FD)
                    nc.sync.dma_start(kt[:, :, sl], k_r[:, :, sl])
                    nc.sync.dma_start(qt[:, :, sl], q_r[:, :, sl])
                    nc.scalar.dma_start(post[:, :, sl], pos_r[:, :, sl])
            else:
                nc.sync.dma_start(kt[:], k_r[:, :, csl])
                nc.sync.dma_start(qt[:], q_r[:, :, csl])
                nc.scalar.dma_start(post[:], pos_r[:, :, csl])
            outt = iopool.tile([P, NDC, CHUNK], BF, tag="outt", bufs=2, name="outt")
            io[ci] = {"kt": kt, "qt": qt, "post": post, "outt": outt}

        def s1(j):
            ci, it = divmod(j, ipc)
            t = io[ci]
            tsl = bass.ts(it, FD)
            h1w = pp.tile([P, FD], F32, tag="h1", bufs=2, name="h1w")
            nc.tensor.matmul(h1w[:], a_t[:, 0, :], t["kt"][:, 0, tsl],
                             start=True, stop=False)
            nc.tensor.matmul(h1w[:], a_t[:, 1, :], t["kt"][:, 1, tsl],
                             start=False, stop=False)
            nc.tensor.matmul(h1w[:], bn_t[:, 0, :], t["qt"][:, 0, tsl],
                             start=False, stop=False)
            nc.tensor.matmul(h1w[:], bn_t[:, 1, :], t["qt"][:, 1, tsl],
                             start=False, stop=False)
            nc.tensor.matmul(h1w[:], w1_t[:, 0, :], t["post"][:, 0, tsl],
                             start=False, stop=False)
            nc.tensor.matmul(h1w[:], w1_t[:, 1, :], t["post"][:, 1, tsl],
                             start=False, stop=True)
            vw = pp.tile([P, NDC, FD], F32, tag="v", bufs=1, name="vw")
            for e in range(NDC):
                esl = bass.ts(e, P)
                nc.tensor.matmul(vw[:, e, :], wkv_t[:, 0, esl], t["kt"][:, 0, tsl],
                                 start=True, stop=False)
                nc.tensor.matmul(vw[:, e, :], wkv_t[:, 1, esl], t["kt"][:, 1, tsl],
                                 start=False, stop=True)
            vp_t = mid.tile([P, NDC, FD], BF, tag="vp", bufs=5, name="vp_t")
            nc.vector.tensor_add(vp_t[:], vw[:], t["post"][:, :, tsl])
            st[j] = {"h1": h1w, "vp": vp_t}

        def s2(j):
            s = st[j]
            h1r = mid.tile([P, FD], BF, tag="h1r", bufs=2, name="h1r")
            nc.scalar.activation(h1r[:], s["h1"][:], AF.Relu, bias=b1_t[:, 0:1])
            h2w = pp.tile([P, NDC, FD], F32, tag="h2", bufs=1, name="h2w")
            for e in range(NDC):
                esl = bass.ts(e, P)
                nc.tensor.matmul(h2w[:, e, :], w2_t[:, esl], h1r[:],
                                 start=True, stop=True)
            s["h2"] = h2w

        def s3(j):
            s = st[j]
            at_t = mid.tile([P, NDC, FD], BF, tag="at", bufs=3, name="at_t")
            for e in range(NDC):
                nc.scalar.activation(at_t[:, e, :], s["h2"][:, e, :], AF.Sigmoid,
                                     bias=b2_t[e][:, 0:1])
            s["at"] = at_t

        def s4(j):
            s = st[j]
            g_t = mid.tile([P, NDC, FD], BF, tag="g", bufs=3, name="g_t")
            nc.gpsimd.tensor_mul(g_t[:, 0, :], s["vp"][:, 0, :], s["at"][:, 0, :])
            nc.vector.tensor_mul(g_t[:, 1, :], s["vp"][:, 1, :], s["at"][:, 1, :])
            s["g"] = g_t

        def s5(j):
            ci, it = divmod(j, ipc)
            t = io[ci]
            tsl = bass.ts(it, FD)
            s = st[j]
            xo = pp.tile([P, NDC, FD], F32, tag="xo", bufs=1, name="xo")
            for e in range(NDC):
                esl = bass.ts(e, P)
                nc.tensor.matmul(xo[:, e, :], wo_t[:, 0, esl], s["g"][:, 0, :],
                                 start=True, stop=False)
                nc.tensor.matmul(xo[:, e, :], wo_t[:, 1, esl], s["g"][:, 1, :],
                                 start=False, stop=True)
            nc.vector.tensor_copy(t["outt"][:, 0, tsl], xo[:, 0, :])
            nc.scalar.copy(t["outt"][:, 1, tsl], xo[:, 1, :])
            del st[j]
            # out DMA: per 1024 tokens; last chunk per 512 to shrink the tail
            base = ci * CHUNK
            if ci == n_chunks - 1:
                nc.sync.dma_start(out_r[:, :, base + it * FD:base + (it + 1) * FD],
                                  t["outt"][:, :, tsl])
            elif it % 2 == 1:
                sl2 = slice((it - 1) * FD, (it + 1) * FD)
                nc.sync.dma_start(out_r[:, :, base + (it - 1) * FD:base + (it + 1) * FD],
                                  t["outt"][:, :, sl2])

        for t in range(n_total + 4):
            if t - 3 >= 0 and t - 3 < n_total:
                s4(t - 3)
            if t < n_total:
                if t % ipc == 0:
                    ci = t // ipc
                    if ci == 0:
                        load_chunk(0)
                        load_chunk(1)
                    elif ci + 1 < n_chunks:
                        load_chunk(ci + 1)
                s1(t)
            if t - 2 >= 0 and t - 2 < n_total:
                s3(t - 2)
            if t - 1 >= 0 and t - 1 < n_total:
                s2(t - 1)
            if t - 4 >= 0 and t - 4 < n_total:
                s5(t - 4)

    nc.compile()
    return nc


def _get_nc():
    global _CACHED_NC
    if _CACHED_NC is None:
        _CACHED_NC = _build_nc()
    return _CACHED_NC


def _prep_in_maps(q, k, pos, Wq, Wk, Wv, W1, b1, W2, b2, Wo, bo):
    q = np.asarray(q, dtype=np.float32)
    k = np.asarray(k, dtype=np.float32)
    pos = np.asarray(pos, dtype=np.float32)
    Wq32 = np.asarray(Wq, np.float32)
    Wk32 = np.asarray(Wk, np.float32)
    W132 = np.asarray(W1, np.float32)

    weights = {
        "wa": np.ascontiguousarray((Wk32 @ W132).astype(BF16)).reshape(NDC, P, P),
        "wbn": np.ascontiguousarray((-(Wq32 @ W132)).astype(BF16)).reshape(NDC, P, P),
        "w1": np.ascontiguousarray(W132.astype(BF16)).reshape(NDC, P, P),
        "wkv": np.ascontiguousarray(
            (Wk32 @ np.asarray(Wv, np.float32)).astype(BF16)
        ).reshape(NDC, P, DIM),
        "w2": np.ascontiguousarray(np.asarray(W2, np.float32).astype(BF16)).reshape(P, DIM),
        "wo": np.ascontiguousarray(np.asarray(Wo, np.float32).astype(BF16)).reshape(NDC, P, DIM),
        "b1": np.asarray(b1, np.float32).reshape(P, 1),
        "b2": np.asarray(b2, np.float32).reshape(NDC, P, 1),
    }

    nhalf = N // 2
    in_maps = []
    for c in range(NCORES):
        b = c // 2
        n0 = (c % 2) * nhalf
        qs = q[b, :, n0:n0 + nhalf, :].reshape(DIM, NT)
        ks = k[b, :, n0:n0 + nhalf, :].reshape(DIM, NT)
        ps = np.ascontiguousarray(
            pos[b, n0:n0 + nhalf].reshape(NT, DIM).T
        ).astype(BF16)
        m = dict(weights)
        m["qs"] = qs.astype(BF16).reshape(NDC, P, NT)
        m["ks"] = ks.astype(BF16).reshape(NDC, P, NT)
        m["poss"] = ps.reshape(NDC, P, NT)
        in_maps.append(m)
    return in_maps


def _run(in_maps, trace=False, **kwargs):
    nc = _get_nc()
    return bass_utils.run_bass_kernel_spmd(
        nc, in_maps, core_ids=list(range(NCORES)), trace=trace, **kwargs
    )


def _assemble(results, bo, mask):
    bo = np.asarray(bo, np.float32)
    out = np.empty((B, DIM, N, M), dtype=np.float32)
    nhalf = N // 2
    for c in range(NCORES):
        b = c // 2
        n0 = (c % 2) * nhalf
        r = results[c]["out"].reshape(DIM, nhalf, M).astype(np.float32)
        r += bo[:, None, None]
        out[b, :, n0:n0 + nhalf, :] = r
    mask = np.asarray(mask)
    if not np.all(mask != 0):
        # masked positions: sigmoid(-1e9)=0 -> x=0 -> out = bo
        zb, zn, zm = np.nonzero(mask[..., 0] == 0)
        out[zb, :, zn, zm] = bo[None, :]
    return out


def kernel(q, k, pos, mask, Wq, Wk, Wv, W1, b1, W2, b2, Wo, bo):
    in_maps = _prep_in_maps(q, k, pos, Wq, Wk, Wv, W1, b1, W2, b2, Wo, bo)
    res = _run(in_maps)
    return _assemble(res.results, bo, mask)


def install_profile_hook():
    """Register the axon NTFF profiling hook (antenv.axon_hooks shim) so
    run_bass_kernel_spmd(trace=True) yields exec_time_ns + perfetto trace."""
    import types

    try:
        import antenv.axon_hooks  # noqa: F401
        return True
    except ImportError:
        pass
    try:
        from trn_agent_boot.trn_boot import _ntff_profile_via_ctypes
    except ImportError:
        return False
    hook = _ntff_profile_via_ctypes("/opt/axon/libaxon_pjrt.so")
    if hook is None:
        return False
    mod = types.ModuleType("antenv.axon_hooks")
    mod.get_axon_ntff_profile_hook = lambda: hook
    mod.set_axon_ntff_profile_hook = lambda h: None
    import antenv

    sys.modules["antenv.axon_hooks"] = mod
    antenv.axon_hooks = mod
    # artifact upload has no share reachable from this container
    bass_utils.upload_artifacts = lambda tmpdir: tmpdir
    return True
